# revision 13
# baseline (speedup 1.0000x reference)
"""Trainium2 Bass kernel for the Mamba U-Net model (nn_Model_20770461843918).

Batch-data-parallel SPMD over 8 NeuronCores (4 batch elements; cores c and
c+4 duplicate work, outputs read from cores 0-3).  Per core the whole
7-block Mamba U-Net runs locally with partitions = inner channel d:
  PE : all matmuls (in/x/dt/out projections, depthwise conv via diagonal
       matmuls, down/up/gate convs) + K=1 ones-matmul broadcast of the
       per-timestep B/C rows across partitions
  ACT: exp(dt*A) per state n, silu, softplus, sigmoid, PSUM->SBUF copies
  DVE: dBu = (dt*u)*B_rep, selective scan via tensor_tensor_scan
       (h_t = dA_t*h_{t-1} + dBu_t, fp32 state), h*C_rep, tree-reduce over n
"""
import numpy as np

B, L0, C = 4, 1024, 128
DI, NST, R, KC = 256, 16, 8, 4
NV = NST + 3          # packed per-partition vec cols: A[16], D, convb, bdt
NCORES = 8
TS = 384              # scan-stage time chunk
MM = 512              # matmul-stage time chunk

_CACHE = {}


def _prep_weights(inp):
    f32 = np.float32
    g = lambda k: np.asarray(inp[k], f32)
    m_Win, m_convw, m_convb = g("m_Win"), g("m_convw"), g("m_convb")
    m_Wx, m_Wdt, m_bdt = g("m_Wx"), g("m_Wdt"), g("m_bdt")
    m_Alog, m_D, m_Wout = g("m_Alog"), g("m_D"), g("m_Wout")
    dc_w, dc_b = g("dc_w"), g("dc_b")
    wg_W, wg_b, db_W, db_b = g("wg_W"), g("wg_b"), g("db_W"), g("db_b")
    up_w, up_b = g("up_w"), g("up_b")

    w = {}
    w["winT"] = np.ascontiguousarray(m_Win.transpose(0, 2, 1))           # [7, C, 512]
    cd = np.zeros((7, 2, KC, 128, 128), f32)
    idx = np.arange(128)
    for i in range(7):
        for gg in range(2):
            for k in range(KC):
                cd[i, gg, k, idx, idx] = m_convw[i, gg * 128:(gg + 1) * 128, k]
    # sbuf layout [128, (g, k, 128)]: partition = k_in, free-block (g,k) = lhsT
    w["convdiag"] = np.ascontiguousarray(cd.transpose(0, 1, 3, 2, 4)).reshape(7, 2, 128, KC * 128)
    wxT_raw = np.ascontiguousarray(m_Wx.transpose(0, 2, 1)).reshape(7, 2, 128, R + 2 * NST)
    wxT = np.zeros((7, 2, 128, 64), f32)
    wxT[..., :R] = wxT_raw[..., :R]          # dt rows -> psum partitions 0..7
    wxT[..., 32:64] = wxT_raw[..., R:]       # B/C rows -> psum partitions 32..63
    w["wxT"] = wxT
    w["wdtT"] = np.ascontiguousarray(m_Wdt.transpose(0, 2, 1))           # [7, R, DI=256]
    A = -np.exp(m_Alog)                                                  # [7, DI, N]
    vec = np.zeros((7, 2, 128, NV), f32)
    for gg in range(2):
        sl = slice(gg * 128, (gg + 1) * 128)
        vec[:, gg, :, :NST] = A[:, sl, :]
        vec[:, gg, :, NST] = m_D[:, sl]
        vec[:, gg, :, NST + 1] = m_convb[:, sl]
        vec[:, gg, :, NST + 2] = m_bdt[:, sl]
    w["vecs"] = vec
    w["woutT"] = np.ascontiguousarray(m_Wout.transpose(0, 2, 1)).reshape(7, 2, 128, C)
    # dc_w[j, co, ci, k] -> [j, ci, (k, co)]
    w["dcwT"] = np.ascontiguousarray(dc_w.transpose(0, 2, 3, 1)).reshape(3, 128, 3 * 128)
    # up_w[j, ci, co, k] -> [j, ci, (k, co)]
    w["upw"] = np.ascontiguousarray(up_w.transpose(0, 1, 3, 2)).reshape(3, 128, 2 * 128)
    w["wgT"] = np.ascontiguousarray(wg_W.transpose(0, 2, 1)).reshape(3, 2, 128, 128)
    w["dbT"] = np.ascontiguousarray(db_W.transpose(0, 2, 1)).reshape(3, 2, 128, 128)
    gv = np.zeros((3, 128, 4), f32)
    gv[:, :, 0], gv[:, :, 1], gv[:, :, 2], gv[:, :, 3] = dc_b, up_b, wg_b, db_b
    w["gvecs"] = gv
    return w


def _build():
    import concourse.bacc as bacc
    import concourse.tile as tile
    import concourse.mybir as mybir

    F32 = mybir.dt.float32
    Alu = mybir.AluOpType
    Act = mybir.ActivationFunctionType

    nc = bacc.Bacc("TRN2", target_bir_lowering=False, debug=False,
                   num_devices=NCORES)

    xT_d = nc.declare_dram_parameter("xT", [C, L0], F32, isOutput=False)
    out_d = nc.declare_dram_parameter("out", [C, L0], F32, isOutput=True)
    dram = {}
    for name, shape in [
        ("winT", [7, C, 2 * DI]), ("convdiag", [7, 2, 128, KC * 128]),
        ("wxT", [7, 2, 128, 64]), ("wdtT", [7, R, DI]),
        ("vecs", [7, 2, 128, NV]), ("woutT", [7, 2, 128, C]),
        ("dcwT", [3, 128, 3 * 128]), ("upw", [3, 128, 2 * 128]),
        ("wgT", [3, 2, 128, 128]), ("dbT", [3, 2, 128, 128]),
        ("gvecs", [3, 128, 4]),
    ]:
        dram[name] = nc.declare_dram_parameter(name, shape, F32, isOutput=False)
    BF16 = mybir.dt.bfloat16
    bc_dram = nc.dram_tensor("bc_bounce", [2 * NST, L0], BF16)

    with tile.TileContext(nc) as tc:
        with tc.tile_pool(name="wt", bufs=1) as wt, \
             tc.tile_pool(name="lvl", bufs=1) as lvl, \
             tc.tile_pool(name="blk", bufs=1) as blk, \
             tc.tile_pool(name="cube", bufs=1) as cube, \
             tc.tile_pool(name="cw", bufs=2) as cw, \
             tc.tile_pool(name="mmp", bufs=2, space="PSUM") as mmp, \
             tc.tile_pool(name="xdbp", bufs=2, space="PSUM") as xdbp, \
             tc.tile_pool(name="repp", bufs=4, space="PSUM") as repp:

            ones2 = wt.tile([65, 128], BF16, tag="ones2")
            nc.vector.memset(ones2[0:1, :], 1.0)
            nc.vector.memset(ones2[64:65, :], 1.0)

            def wload(name, i, shape, tag, parts=None):
                t = wt.tile(shape, F32, tag=tag)
                if parts is None:
                    nc.sync.dma_start(t[:], dram[name][i])
                else:
                    for pi, (dst, src) in enumerate(parts):
                        nc.sync.dma_start(dst(t), src)
                return t

            winT, wxTt, wdtTt, vecst, woutTt = [], [], [], [], []
            for i in range(7):
                winT.append(wload("winT", i, [C, 2 * DI], f"winT{i}"))
                t = wt.tile([128, 2 * 64], F32, tag=f"wxT{i}")
                nc.sync.dma_start(t[:, :64], dram["wxT"][i, 0])
                nc.sync.dma_start(t[:, 64:], dram["wxT"][i, 1])
                wxTt.append(t)
                wdtTt.append(wload("wdtT", i, [R, DI], f"wdtT{i}"))
                t = wt.tile([128, 2 * NV], F32, tag=f"vecs{i}")
                nc.sync.dma_start(t[:, :NV], dram["vecs"][i, 0])
                nc.sync.dma_start(t[:, NV:], dram["vecs"][i, 1])
                vecst.append(t)
                t = wt.tile([128, 2 * C], F32, tag=f"woutT{i}")
                nc.sync.dma_start(t[:, :C], dram["woutT"][i, 0])
                nc.sync.dma_start(t[:, C:], dram["woutT"][i, 1])
                woutTt.append(t)
            dcwTt, upwt, wgTt, dbTt, gvecst = [], [], [], [], []
            for j in range(3):
                dcwTt.append(wload("dcwT", j, [128, 3 * 128], f"dcwT{j}"))
                upwt.append(wload("upw", j, [128, 2 * 128], f"upw{j}"))
                t = wt.tile([128, 2 * 128], F32, tag=f"wgT{j}")
                nc.sync.dma_start(t[:, :128], dram["wgT"][j, 0])
                nc.sync.dma_start(t[:, 128:], dram["wgT"][j, 1])
                wgTt.append(t)
                t = wt.tile([128, 2 * 128], F32, tag=f"dbT{j}")
                nc.sync.dma_start(t[:, :128], dram["dbT"][j, 0])
                nc.sync.dma_start(t[:, 128:], dram["dbT"][j, 1])
                dbTt.append(t)
                gvecst.append(wload("gvecs", j, [128, 4], f"gvecs{j}"))

            # per-block working tiles (reused across blocks)
            xi = [blk.tile([128, L0 + 3], F32, tag=f"xi{g}", name=f"xi{g}")
                  for g in range(2)]
            u_t = [blk.tile([128, L0], F32, tag=f"u{g}", name=f"u{g}")
                   for g in range(2)]
            dt_t = [blk.tile([128, L0], F32, tag=f"dt{g}", name=f"dt{g}")
                    for g in range(2)]
            y_t = [blk.tile([128, L0], F32, tag=f"y{g}", name=f"y{g}")
                   for g in range(2)]
            xdbR = blk.tile([R, L0], F32, tag="xdbR")
            bc16 = blk.tile([2 * NST, L0], BF16, tag="bc16")
            carry = blk.tile([128, 2 * NST], F32, tag="carry")
            ztmp = blk.tile([128, MM], F32, tag="ztmp")
            dA_t = cube.tile([128, NST * TS], F32, tag="dA")
            dBu_t = cube.tile([128, NST * TS], F32, tag="dBu")

            def mamba(x_ap, i, Lb, out_ap):
                cdw = cw.tile([128, 2 * KC * 128], F32, tag="convdiag")
                nc.sync.dma_start(cdw[:, :KC * 128], dram["convdiag"][i, 0])
                nc.sync.dma_start(cdw[:, KC * 128:], dram["convdiag"][i, 1])
                vecs = vecst[i]

                def vcol(g, c):
                    return vecs[:, g * NV + c: g * NV + c + 1]

                for g in range(2):
                    nc.vector.memset(xi[g][:, :3], 0.0)
                # ---- stage M ----
                for c0 in range(0, Lb, MM):
                    F = min(MM, Lb - c0)
                    for p in range(2):
                        ps = mmp.tile([128, MM], F32, tag="mmps")
                        nc.tensor.matmul(ps[:, :F], winT[i][:, p * 128:(p + 1) * 128],
                                         x_ap[:, c0:c0 + F], start=True, stop=True)
                        nc.scalar.activation(xi[p][:, 3 + c0:3 + c0 + F], ps[:, :F], Act.Copy)
                    for g in range(2):
                        ps = mmp.tile([128, MM], F32, tag="mmps")
                        for k in range(KC):
                            nc.tensor.matmul(
                                ps[:, :F],
                                cdw[:, (g * KC + k) * 128:(g * KC + k + 1) * 128],
                                xi[g][:, c0 + k:c0 + k + F],
                                start=(k == 0), stop=(k == KC - 1))
                        nc.scalar.activation(u_t[g][:, c0:c0 + F], ps[:, :F], Act.Identity,
                                             bias=vcol(g, NST + 1))
                        nc.scalar.activation(ztmp[:, :F], ps[:, :F], Act.Sigmoid,
                                             bias=vcol(g, NST + 1))
                        nc.vector.tensor_mul(u_t[g][:, c0:c0 + F], u_t[g][:, c0:c0 + F],
                                             ztmp[:, :F])
                    psx = xdbp.tile([64, MM], F32, tag="xdbps")
                    for g in range(2):
                        nc.tensor.matmul(psx[:, :F],
                                         wxTt[i][:, g * 64:(g + 1) * 64],
                                         u_t[g][:, c0:c0 + F], start=(g == 0), stop=(g == 1))
                    nc.scalar.activation(xdbR[:, c0:c0 + F], psx[:R, :F], Act.Copy)
                    nc.scalar.activation(bc16[:, c0:c0 + F], psx[32:, :F], Act.Copy)
                    for g in range(2):
                        ps = mmp.tile([128, MM], F32, tag="mmps")
                        nc.tensor.matmul(ps[:, :F], wdtTt[i][:, g * 128:(g + 1) * 128],
                                         xdbR[:, c0:c0 + F], start=True, stop=True)
                        nc.scalar.activation(ztmp[:, :F], ps[:, :F], Act.Exp,
                                             bias=vcol(g, NST + 2))
                        nc.scalar.activation(dt_t[g][:, c0:c0 + F], ztmp[:, :F], Act.Ln,
                                             bias=1.0)
                nc.sync.dma_start(bc_dram[:, :Lb], bc16[:, :Lb])
                # ---- stage S ----
                nchunks = (Lb + TS - 1) // TS
                for s in range(nchunks):
                    s0 = s * TS
                    F = min(TS, Lb - s0)
                    bcz = cw.tile([65, NST * TS], BF16, tag="bcz")
                    nc.sync.dma_start(bcz[0:1, :NST * F], bc_dram[0:NST, s0:s0 + F])
                    nc.sync.dma_start(bcz[64:65, :NST * F], bc_dram[NST:, s0:s0 + F])
                    for g in range(2):
                        dtu = cw.tile([128, TS], F32, tag="dtu")
                        nc.vector.tensor_mul(dtu[:, :F], dt_t[g][:, s0:s0 + F],
                                             u_t[g][:, s0:s0 + F])
                        for n in range(NST):
                            nc.scalar.activation(dA_t[:, n * F:(n + 1) * F],
                                                 dt_t[g][:, s0:s0 + F], Act.Exp,
                                                 scale=vcol(g, n))
                        for n in range(NST):
                            rep = repp.tile([128, TS], F32, tag="rep")
                            nc.tensor.matmul(rep[:, :F], ones2[0:1, :],
                                             bcz[0:1, n * F:(n + 1) * F],
                                             start=True, stop=True)
                            nc.vector.tensor_mul(dBu_t[:, n * F:(n + 1) * F],
                                                 dtu[:, :F], rep[:, :F])
                        for n in range(NST):
                            init = 0.0 if s == 0 else carry[:, g * NST + n:g * NST + n + 1]
                            nc.vector.tensor_tensor_scan(
                                dBu_t[:, n * F:(n + 1) * F],
                                dA_t[:, n * F:(n + 1) * F],
                                dBu_t[:, n * F:(n + 1) * F],
                                init, op0=Alu.mult, op1=Alu.add)
                        if s + 1 < nchunks:
                            nc.vector.tensor_copy(carry[:, g * NST:(g + 1) * NST],
                                                  dBu_t[:, F - 1:NST * F:F])
                        for n in range(NST):
                            rep = repp.tile([128, TS], F32, tag="rep")
                            nc.tensor.matmul(rep[:, :F], ones2[64:65, :],
                                             bcz[64:65, n * F:(n + 1) * F],
                                             start=True, stop=True)
                            nc.vector.tensor_mul(dA_t[:, n * F:(n + 1) * F],
                                                 dBu_t[:, n * F:(n + 1) * F], rep[:, :F])
                        nc.vector.tensor_add(dA_t[:, :8 * F], dA_t[:, :8 * F], dA_t[:, 8 * F:16 * F])
                        nc.vector.tensor_add(dA_t[:, :4 * F], dA_t[:, :4 * F], dA_t[:, 4 * F:8 * F])
                        nc.vector.tensor_add(dA_t[:, :2 * F], dA_t[:, :2 * F], dA_t[:, 2 * F:4 * F])
                        nc.vector.tensor_add(y_t[g][:, s0:s0 + F], dA_t[:, :F], dA_t[:, F:2 * F])
                # ---- stage O ----
                for c0 in range(0, Lb, MM):
                    F = min(MM, Lb - c0)
                    for g in range(2):
                        nc.vector.scalar_tensor_tensor(
                            y_t[g][:, c0:c0 + F], u_t[g][:, c0:c0 + F], vcol(g, NST),
                            y_t[g][:, c0:c0 + F], op0=Alu.mult, op1=Alu.add)
                        ps = mmp.tile([128, MM], F32, tag="mmps")
                        nc.tensor.matmul(ps[:, :F], winT[i][:, (2 + g) * 128:(3 + g) * 128],
                                         x_ap[:, c0:c0 + F], start=True, stop=True)
                        nc.scalar.activation(ztmp[:, :F], ps[:, :F], Act.Sigmoid)
                        nc.vector.tensor_mul(y_t[g][:, c0:c0 + F], y_t[g][:, c0:c0 + F],
                                             ztmp[:, :F])
                        nc.scalar.activation(ztmp[:, :F], ps[:, :F], Act.Copy)
                        nc.vector.tensor_mul(y_t[g][:, c0:c0 + F], y_t[g][:, c0:c0 + F],
                                             ztmp[:, :F])
                    ps = mmp.tile([128, MM], F32, tag="mmps")
                    for g in range(2):
                        nc.tensor.matmul(ps[:, :F], woutTt[i][:, g * C:(g + 1) * C],
                                         y_t[g][:, c0:c0 + F], start=(g == 0), stop=(g == 1))
                    nc.scalar.activation(out_ap[:, c0:c0 + F], ps[:, :F], Act.Copy)

            def downconv(xt, off, j, Lb, out_ap):
                """xt: level tile; data at cols [off, off+Lb); front pad col off-1."""
                Lo = Lb // 2
                for c0 in range(0, Lo, MM):
                    F = min(MM, Lo - c0)
                    ps = mmp.tile([128, MM], F32, tag="mmps")
                    for k in range(3):
                        a = off + 2 * c0 + k - 1
                        nc.tensor.matmul(ps[:, :F], dcwTt[j][:, k * 128:(k + 1) * 128],
                                         xt[:, a:a + 2 * F - 1:2],
                                         start=(k == 0), stop=(k == 2))
                    nc.scalar.activation(out_ap[:, c0:c0 + F], ps[:, :F], Act.Identity,
                                         bias=gvecst[j][:, 0:1])

            def gate(t1_ap, t2_ap, j, Lb, f_ap):
                Fh = MM // 2
                for c0 in range(0, Lb, MM):   # output chunk
                    F = min(MM, Lb - c0)
                    ch = c0 // 2
                    Fi = F // 2
                    t2u = cw.tile([128, MM], F32, tag="t2u")
                    pse = mmp.tile([128, MM], F32, tag="mmps")
                    nc.tensor.matmul(pse[:, :Fi], upwt[j][:, :128],
                                     t2_ap[:, ch:ch + Fi], start=True, stop=True)
                    nc.scalar.activation(t2u[:, 0:F:2], pse[:, :Fi], Act.Identity,
                                         bias=gvecst[j][:, 1:2])
                    pso = mmp.tile([128, MM], F32, tag="mmps")
                    nc.tensor.matmul(pso[:, :Fi], upwt[j][:, 128:],
                                     t2_ap[:, ch:ch + Fi], start=True, stop=True)
                    nc.scalar.activation(t2u[:, 1:F:2], pso[:, :Fi], Act.Identity,
                                         bias=gvecst[j][:, 1:2])
                    ps = mmp.tile([128, MM], F32, tag="mmps")
                    nc.tensor.matmul(ps[:, :F], wgTt[j][:, :128], t1_ap[:, c0:c0 + F],
                                     start=True, stop=False)
                    nc.tensor.matmul(ps[:, :F], wgTt[j][:, 128:], t2u[:, :F],
                                     start=False, stop=True)
                    wloc = cw.tile([128, MM], F32, tag="wloc")
                    nc.scalar.activation(wloc[:, :F], ps[:, :F], Act.Sigmoid,
                                         bias=gvecst[j][:, 2:3])
                    m1 = cw.tile([128, MM], F32, tag="m1")
                    m2 = cw.tile([128, MM], F32, tag="m2")
                    nc.vector.tensor_mul(m1[:, :F], t1_ap[:, c0:c0 + F], wloc[:, :F])
                    nc.vector.tensor_mul(m2[:, :F], t2u[:, :F], wloc[:, :F])
                    nc.vector.tensor_sub(m2[:, :F], t2u[:, :F], m2[:, :F])
                    ps2 = mmp.tile([128, MM], F32, tag="mmps")
                    nc.tensor.matmul(ps2[:, :F], dbTt[j][:, :128], m1[:, :F],
                                     start=True, stop=False)
                    nc.tensor.matmul(ps2[:, :F], dbTt[j][:, 128:], m2[:, :F],
                                     start=False, stop=True)
                    nc.scalar.activation(f_ap[:, c0:c0 + F], ps2[:, :F], Act.Identity,
                                         bias=gvecst[j][:, 3:4])

            # ---------- network ----------
            x1 = lvl.tile([128, 1025], F32, tag="x1")
            x2 = lvl.tile([128, 513], F32, tag="x2")
            x3 = lvl.tile([128, 257], F32, tag="x3")
            x4 = lvl.tile([128, 128], F32, tag="x4")
            e1 = lvl.tile([128, 1024], F32, tag="e1")
            e2 = lvl.tile([128, 512], F32, tag="e2")
            e3 = lvl.tile([128, 256], F32, tag="e3")
            e4 = lvl.tile([128, 128], F32, tag="e4")
            d4 = lvl.tile([128, 256], F32, tag="d4")
            d3 = lvl.tile([128, 512], F32, tag="d3")
            fbuf = lvl.tile([128, 1024], F32, tag="fbuf")

            nc.vector.memset(x1[:, 0:1], 0.0)
            nc.vector.memset(x2[:, 0:1], 0.0)
            nc.vector.memset(x3[:, 0:1], 0.0)
            nc.sync.dma_start(x1[:, 1:1025], xT_d[:, :])

            mamba(x1[:, 1:1025], 0, 1024, e1[:, :])
            downconv(x1, 1, 0, 1024, x2[:, 1:513])
            mamba(x2[:, 1:513], 1, 512, e2[:, :])
            downconv(x2, 1, 1, 512, x3[:, 1:257])
            mamba(x3[:, 1:257], 2, 256, e3[:, :])
            downconv(x3, 1, 2, 256, x4[:, :])
            mamba(x4[:, :], 3, 128, e4[:, :])
            gate(e3[:, :], e4[:, :], 0, 256, fbuf[:, :256])
            mamba(fbuf[:, :256], 4, 256, d4[:, :])
            gate(e2[:, :], d4[:, :], 1, 512, fbuf[:, :512])
            mamba(fbuf[:, :512], 5, 512, d3[:, :])
            gate(e1[:, :], d3[:, :], 2, 1024, fbuf[:, :])
            d2 = x1  # x1 dead by now; reuse its slot
            mamba(fbuf[:, :], 6, 1024, d2[:, 1:1025])
            nc.sync.dma_start(out_d[:, :], d2[:, 1:1025])

    nc.compile()
    return nc


def _get_program():
    if "nc" not in _CACHE:
        _CACHE["nc"] = _build()
    return _CACHE["nc"]


def kernel(**inputs):
    from concourse.bass_utils import run_bass_kernel_spmd

    nc = _get_program()
    w = _prep_weights(inputs)
    x = np.asarray(inputs["x"], np.float32)  # [B, L, C]
    in_maps = []
    for c in range(NCORES):
        m = {"xT": np.ascontiguousarray(x[c % B].T)}
        m.update(w)
        in_maps.append(m)
    res = run_bass_kernel_spmd(nc, in_maps, list(range(NCORES)))
    out = np.empty((B, L0, C), np.float32)
    for b in range(B):
        out[b] = res.results[b]["out"].T
    return out
